# revision 41
# baseline (speedup 1.0000x reference)
"""AttentionBlock (GroupNorm + single-head 4096x4096 attention + proj + residual)
on 8 Trainium2 NeuronCores.

Sharding: core c = 2*b + h handles image b (of 4), query-half h (of 2).
Each core:
  - receives x pre-transposed to channel-major bf16 (host-side, free),
  - GroupNorm statistics via bn_stats while x streams in,
  - computes kT/v for the full image and qT for its 2048 query rows
    (bf16 GEMMs, outputs quantized to fp8e4m3),
  - attention over its 2048 queries in fp8 DoubleRow matmuls (2x PE
    throughput).  Softmax without max subtraction; exp is computed as
    exp(s*scale - 3) so the fp8 et values stay within e4m3 range
    (scores are within +-7, e4m3 max finite is 240); the -3 offset
    cancels in the num/den division,
  - projection (bf16) + bias + residual for its rows.
No collectives; k/v compute is duplicated across the half-pair (~10% FLOPs).

Precision: bf16 GEMMs for hn/q/k/v/proj with fp32 PSUM accumulation;
fp8e4m3 operands for scores/PV/rowsum GEMMs; fp32 GroupNorm statistics,
fp32 softmax row-sums / normalization, fp32 residual.  Measured ~4e-3
max rel err vs fp32 (gate is 2e-2).
"""

import sys

sys.path.insert(0, "/opt/trn_rl_repo")

import numpy as np  # noqa: E402

import bass_rust  # noqa: E402
import concourse.bass as bass  # noqa: E402
import concourse.mybir as mybir  # noqa: E402
import concourse.tile as tile  # noqa: E402
from concourse.vector_clock import ScopedClock  # noqa: E402
from concourse.bass_utils import run_bass_kernel_spmd  # noqa: E402

F32 = mybir.dt.float32
BF16 = mybir.dt.bfloat16
F8 = mybir.dt.float8e4
AF = mybir.ActivationFunctionType
OP = mybir.AluOpType
DR = mybir.MatmulPerfMode.DoubleRow

B, H, W, C = 4, 64, 64, 512
HW = H * W            # 4096 positions per image
HALF = HW // 2        # 2048 query rows per core
GROUPS = 32
GSIZE = C // GROUPS   # 16 channels per group
EPS = 1e-5
SM_SCALE = 1.0 / float(np.sqrt(C))
EB = 3.0              # exp bias: et = exp(s*SM - EB), cancels in num/den
N_CORES = 8
FP8_B = True          # fp8 DoubleRow GEMMs in phase B (else bf16)
FP8_P = True          # fp8 DoubleRow projection (else bf16)
CT = C // 128         # 4 channel partition-tiles
JT = HW // 128        # 32 position partition-tiles
JC = HW // 512        # 8 position chunks (kT/v build)
QC = HALF // 512      # 4 query chunks (qT build)
IB = HALF // 512      # 4 query i-blocks (attention)
JP = JT // 2          # 16 key-tile pairs (fp8 DoubleRow)


# --- workaround: walrus in this container rejects instructions carrying more
# than one sync-wait command.  Move extra waits onto same-engine NOPs placed
# immediately before the instruction (engine program order makes this exact).
def _split_multi_waits(nc, max_waits=1):
    n = 0
    for f in nc.m.functions:
        for bb in f.blocks:
            newlist = []
            for inst in bb.instructions:
                si = inst.sync_info
                waits = list(si.on_wait) if si is not None else []
                if len(waits) > max_waits:
                    n += 1
                    for k, wt in enumerate(waits[:-max_waits]):
                        nop = bass_rust.InstNoOp(
                            name=f"{inst.name}-sw{k}", engine=inst.engine)
                        nop.sync_info = mybir.SyncInfo(on_wait=[wt], on_update=[])
                        newlist.append(nop)
                    inst.sync_info = mybir.SyncInfo(
                        on_wait=waits[-max_waits:], on_update=list(si.on_update))
                newlist.append(inst)
            bb.instructions[:] = newlist
    return n


def _split_drain_and_barrier(self, tick_clock, wait_clock):
    # same as TileContext._drain_and_barrier but with the tail drain's waits
    # split onto single-wait NOPs (same walrus limitation as above).
    drain_inst = self.nc.sync.drain()
    wait_clock.add_sem_waits(
        drain_inst.ins, ScopedClock({None: tick_clock.global_clock}))
    mi = drain_inst.ins
    waits = list(mi.sync_info.on_wait) if mi.sync_info is not None else []
    if len(waits) > 1:
        mi.sync_info.on_wait = []
        for wt in waits:
            wi = self.nc.sync.nop(nofuse=True, hint="tail_drain_wait")
            wi.ins.sync_info = mybir.SyncInfo(on_wait=[wt], on_update=[])
    self.nc.all_engine_barrier()
    assert self.sems is not None
    popped = self.nc._tile_sem_poison_stack.pop()
    assert popped is self._sem_poison
    self.nc.clear_and_free_semaphores(list(self.sems.allocated().values()))
    self.nc.all_engine_barrier()


tile.TileContext._drain_and_barrier = _split_drain_and_barrier


def build_program():
    nc = bass.Bass()

    # xt is channel-major bf16, pre-transposed on the host: xt[ct, p, j] =
    # x[j, ct*128+p] with rows permuted per-core so the query half is always
    # positions [0, HALF).  Attention is position-order invariant over keys.
    xtd = nc.dram_tensor("xt", [CT, 128, HW], BF16, kind="ExternalInput")
    xq = nc.dram_tensor("xq", [HALF, C], F32, kind="ExternalInput")
    # weights host-cast in [cin%128, cin//128, cout] layout
    WB = F8 if FP8_B else BF16
    wqd = nc.dram_tensor("wq", [128, CT, C], WB, kind="ExternalInput")
    wkd = nc.dram_tensor("wk", [128, CT, C], WB, kind="ExternalInput")
    wvd = nc.dram_tensor("wv", [128, CT, C], WB, kind="ExternalInput")
    wpd = nc.dram_tensor("wp", [128, CT, C], F8 if FP8_P else BF16, kind="ExternalInput")
    bqd = nc.dram_tensor("bq", [C, 1], F32, kind="ExternalInput")
    bkd = nc.dram_tensor("bk", [C, 1], F32, kind="ExternalInput")
    # bv is folded into bp on the host: softmax rows sum to 1, so
    # attn @ (v + bv) = attn @ v + bv, and (o + bv) @ wp + bp =
    # o @ wp + (bv @ wp + bp).
    bpd = nc.dram_tensor("bp", [C], F32, kind="ExternalInput")
    gamd = nc.dram_tensor("gamma", [C, 1], F32, kind="ExternalInput")
    betd = nc.dram_tensor("beta", [C, 1], F32, kind="ExternalInput")
    gseld = nc.dram_tensor("gsel", [GROUPS, C], F32, kind="ExternalInput")
    # gsel2[p, ct, g] = 1/(GSIZE*HW) where channel ct*128+p belongs to group g
    gsel2d = nc.dram_tensor("gsel2", [128, CT, GROUPS], BF16, kind="ExternalInput")
    yd = nc.dram_tensor("y", [HALF, C], F32, kind="ExternalOutput")

    xqt = xq[:, :].rearrange("(t p) c -> t p c", p=128)   # [16,128,512]
    yt = yd[:, :].rearrange("(t p) c -> t p c", p=128)    # [16,128,512]

    with tile.TileContext(nc) as tc:
        # ---------------- persistent storage + constants ----------------
        store = tc.alloc_tile_pool(name="store", bufs=1)
        kT = store.tile([128, CT, HW], F8)        # kT[c%128, c//128, j]
        vS = store.tile([128, JT, C], F8)         # v[j%128, j//128, c]
        qT = store.tile([128, CT, HALF], F8)      # qT[c%128, c//128, i]
        # x^T in bf16, two tiles per channel-tile (query half first); tile
        # granularity matches the DMA so consumers start per-chunk
        xTs = [[store.tile([128, HW // 2], BF16, tag=f"xT{ct}_{t}",
                           name=f"xT{ct}_{t}") for t in range(2)]
               for ct in range(CT)]
        WB_SB = F8 if FP8_B else BF16
        wq_sb = store.tile([128, CT, C], WB_SB)
        wk_sb = store.tile([128, CT, C], WB_SB)
        wv_sb = store.tile([128, CT, C], WB_SB)
        wp_sb = store.tile([128, CT, C], F8 if FP8_P else BF16)
        cst = tc.alloc_tile_pool(name="cst", bufs=1)
        gsel = cst.tile([GROUPS, C], F32)
        # gsel2 in bf16: its value 1/(GSIZE*HW) = 2^-16 is exact in bf16
        gsel2 = cst.tile([128, CT, GROUPS], BF16)
        # gsel2 is needed by the first phase-A matmuls; everything else
        # streams after x so the x DMAs own the head of both queues
        nc.gpsimd.dma_start(out=gsel2, in_=gsel2d[:, :, :])
        ones3 = cst.tile([128, 2, 128], F8)
        nc.vector.memset(ones3, 1.0)
        ebias = cst.tile([128, 1], F32)
        nc.vector.memset(ebias, -EB)
        bq_sb = cst.tile([128, CT], F32)
        bk_sb = cst.tile([128, CT], F32)
        gam_sb = cst.tile([128, CT], F32)
        bet_sb = cst.tile([128, CT], F32)
        bp_bc = cst.tile([128, C], F32)
        s_sb = cst.tile([128, CT], F32)   # GN scale per channel
        t_sb = cst.tile([128, CT], F32)   # GN shift per channel

        def late_const_dmas():
            for ct in range(CT):
                nc.sync.dma_start(out=bq_sb[:, ct:ct + 1], in_=bqd[ct * 128:(ct + 1) * 128, :])
                nc.sync.dma_start(out=bk_sb[:, ct:ct + 1], in_=bkd[ct * 128:(ct + 1) * 128, :])
                nc.gpsimd.dma_start(out=gam_sb[:, ct:ct + 1], in_=gamd[ct * 128:(ct + 1) * 128, :])
                nc.gpsimd.dma_start(out=bet_sb[:, ct:ct + 1], in_=betd[ct * 128:(ct + 1) * 128, :])
            nc.gpsimd.dma_start(out=gsel, in_=gseld[:, :])
            nc.sync.dma_start(out=bp_bc, in_=bpd[:].partition_broadcast(128))

        # -------- phase A: DMA x^T (host-transposed) + GroupNorm stats ------
        # Group sums of x and x^2 are computed on the (otherwise idle) PE:
        # gsel2^T @ xT accumulates [group, position-chunk] partial sums in
        # PSUM (position chunks fold into the same 512 columns), and the
        # accum_out of the PSUM eviction does the final column reduction.
        # The DVE squares each x tile; gsel2 carries the 1/(GSIZE*HW).
        with tc.tile_pool(name="pa_ps", bufs=1, space="PSUM") as pa_ps, \
             tc.tile_pool(name="pa_bc", bufs=2, space="PSUM") as pa_bc, \
             tc.tile_pool(name="pa_scr", bufs=2) as pa_scr, \
             tc.tile_pool(name="pa_small", bufs=1) as pas:
            gx = pa_ps.tile([GROUPS, 512], F32, tag="gx")
            gx2 = pa_ps.tile([GROUPS, 512], F32, tag="gx2")
            mv2 = pas.tile([GROUPS, 2], F32)
            # arrival order: sync streams ct0,ct1; gpsimd streams ct2,ct3
            order = [(0, 0), (2, 0), (0, 1), (2, 1), (1, 0), (3, 0), (1, 1), (3, 1)]
            for ct, t in order:
                eng = nc.sync if ct < 2 else nc.gpsimd
                eng.dma_start(out=xTs[ct][t],
                              in_=xtd[ct, :, t * 2048:(t + 1) * 2048])
            # weights + small consts queue behind x on both engines
            for i, (wd, wsb) in enumerate(
                    ((wqd, wq_sb), (wkd, wk_sb), (wvd, wv_sb), (wpd, wp_sb))):
                (nc.sync, nc.gpsimd)[i % 2].dma_start(out=wsb, in_=wd[:, :, :])
            late_const_dmas()
            prev = None
            for idx, (ct, t) in enumerate(order):
                for c4 in range(4):
                    nc.tensor.matmul(
                        gx[:, :], gsel2[:, ct, :],
                        xTs[ct][t][:, c4 * 512:(c4 + 1) * 512],
                        start=(idx == 0 and c4 == 0),
                        stop=(idx == 7 and c4 == 3))
                x2 = pa_scr.tile([128, 2048], BF16, tag="x2")
                nc.vector.tensor_tensor(x2[:, :], xTs[ct][t], xTs[ct][t], OP.mult)
                if prev is not None:
                    pct, px2, pidx = prev
                    for c4 in range(4):
                        nc.tensor.matmul(
                            gx2[:, :], gsel2[:, pct, :],
                            px2[:, c4 * 512:(c4 + 1) * 512],
                            start=(pidx == 0 and c4 == 0), stop=False)
                prev = (ct, x2, idx)
            pct, px2, pidx = prev
            for c4 in range(4):
                nc.tensor.matmul(gx2[:, :], gsel2[:, pct, :],
                                 px2[:, c4 * 512:(c4 + 1) * 512],
                                 start=False, stop=(c4 == 3))
            # evict with accum_out: mv2 = (E[x], E[x^2]) per group
            scr32 = pas.tile([GROUPS, 2, 512], F32)
            nc.scalar.activation(scr32[:, 0, :], gx[:, :], AF.Copy,
                                 accum_out=mv2[:, 0:1])
            nc.scalar.activation(scr32[:, 1, :], gx2[:, :], AF.Copy,
                                 accum_out=mv2[:, 1:2])
            var = pas.tile([GROUPS, 1], F32)
            nc.vector.tensor_mul(var[:, :], mv2[:, 0:1], mv2[:, 0:1])
            nc.vector.tensor_sub(var[:, :], mv2[:, 1:2], var[:, :])
            epst = pas.tile([GROUPS, 1], F32)
            nc.vector.memset(epst, EPS)
            sd = pas.tile([GROUPS, 1], F32)
            nc.scalar.activation(sd[:, :], var[:, :], AF.Sqrt, bias=epst[:, :])
            nc.vector.reciprocal(mv2[:, 1:2], sd[:, :])
            # broadcast group (mean, rstd) to channels, then s/t (batched)
            bc_all = pas.tile([128, CT, 2], F32)
            for ct in range(CT):
                pbc = pa_bc.tile([128, 2], F32, tag="bcast")
                nc.tensor.matmul(pbc[:, :], gsel[:, ct * 128:(ct + 1) * 128],
                                 mv2[:, :], start=True, stop=True)
                nc.scalar.activation(bc_all[:, ct, :], pbc[:, :], AF.Copy)
            nc.vector.tensor_mul(s_sb[:, :], gam_sb[:, :], bc_all[:, :, 1])
            tmp = pas.tile([128, CT], F32)
            nc.vector.tensor_mul(tmp[:, :], bc_all[:, :, 0], s_sb[:, :])
            nc.vector.tensor_sub(t_sb[:, :], bet_sb[:, :], tmp[:, :])

        # ---------------- phase B: normalize + K,V (and Q) GEMMs ------------
        # FP8_B: fp8 DoubleRow GEMMs; else bf16 with 4-step contraction.
        def bmm(out, lhs_tile, lhs_cols, rhs_tile, rhs_cols):
            if FP8_B:
                for k in range(2):
                    nc.tensor.matmul(
                        out, lhs_tile[:, 2 * k:2 * k + 2, lhs_cols],
                        rhs_tile[:, 2 * k:2 * k + 2, rhs_cols],
                        start=(k == 0), stop=(k == 1), perf_mode=DR)
            else:
                for k in range(CT):
                    nc.tensor.matmul(
                        out, lhs_tile[:, k, lhs_cols], rhs_tile[:, k, rhs_cols],
                        start=(k == 0), stop=(k == CT - 1))

        def qkv_chunk(pb, pb_ps, jc):
            hnT = pb.tile([128, CT, 512], F8 if FP8_B else BF16, tag="hnT")
            for ct in range(CT):
                # hnT = s * xT + t  (per-channel; channels on partitions)
                nc.vector.tensor_scalar(
                    hnT[:, ct, :],
                    xTs[ct][jc // 4][:, (jc % 4) * 512:(jc % 4 + 1) * 512],
                    s_sb[:, ct:ct + 1], t_sb[:, ct:ct + 1], OP.mult, OP.add)
            for ct in range(CT):
                pk = pb_ps.tile([128, 512], F32, tag="qkv")
                bmm(pk[:, :], wk_sb, slice(ct * 128, (ct + 1) * 128),
                    hnT, slice(None))
                nc.scalar.activation(
                    kT[:, ct, jc * 512:(jc + 1) * 512], pk[:, :],
                    AF.Identity, bias=bk_sb[:, ct:ct + 1])
            if jc < QC:   # rows [0, HALF) are the query rows
                for ct in range(CT):
                    pq = pb_ps.tile([128, 512], F32, tag="qkv")
                    bmm(pq[:, :], wq_sb, slice(ct * 128, (ct + 1) * 128),
                        hnT, slice(None))
                    nc.scalar.activation(
                        qT[:, ct, jc * 512:(jc + 1) * 512], pq[:, :],
                        AF.Identity, bias=bq_sb[:, ct:ct + 1])
            for jp in range(4):
                pv = pb_ps.tile([128, 512], F32, tag="qkv")
                bmm(pv[:, :], hnT, slice(jp * 128, (jp + 1) * 128),
                    wv_sb, slice(None))
                # bv is folded into bp on the host (softmax rows sum to 1)
                nc.vector.tensor_copy(vS[:, jc * 4 + jp, :], pv[:, :])

        with tc.tile_pool(name="pb_sb", bufs=3) as pb, \
             tc.tile_pool(name="pb_ps", bufs=6, space="PSUM") as pb_ps:
            for jc in range(JC):
                qkv_chunk(pb, pb_ps, jc)

        # ---------------- phase C: attention + projection + residual --------
        # scores/PV/rowsum in fp8 DoubleRow (2 k-tiles per matmul, 2x rate)
        with tc.tile_pool(name="pc_sb", bufs=4) as pcs, \
             tc.tile_pool(name="pc_et", bufs=2) as pce, \
             tc.tile_pool(name="pc_res", bufs=2) as pcr, \
             tc.tile_pool(name="pc_o", bufs=2) as pco, \
             tc.tile_pool(name="ps_s", bufs=2, space="PSUM") as ps_s, \
             tc.tile_pool(name="ps_o", bufs=1, space="PSUM") as ps_o, \
             tc.tile_pool(name="ps_r", bufs=1, space="PSUM") as ps_r, \
             tc.tile_pool(name="ps_y", bufs=1, space="PSUM") as ps_y:
            # Software-pipelined: scores/exp for key-pair jp are emitted one
            # iteration ahead of the PV/rowsum that consume them, so the PE
            # never waits on the scalar engine's exp.  The projection of
            # i-block ib-1 is deferred into ib's jp=2 slot so the normalize
            # (vector) runs while the PE streams the next block's scores.
            def scores_block(ib, jp):
                et = pce.tile([128, 2, 512], F8, tag="exp")
                for jj in range(2):
                    j = 2 * jp + jj
                    pss = ps_s.tile([128, 512], F32, tag="scores")
                    nc.tensor.matmul(
                        pss[:, :], kT[:, 0:2, j * 128:(j + 1) * 128],
                        qT[:, 0:2, ib * 512:(ib + 1) * 512],
                        start=True, stop=False, perf_mode=DR)
                    nc.tensor.matmul(
                        pss[:, :], kT[:, 2:4, j * 128:(j + 1) * 128],
                        qT[:, 2:4, ib * 512:(ib + 1) * 512],
                        start=False, stop=True, perf_mode=DR)
                    nc.scalar.activation(et[:, jj, :], pss[:, :], AF.Exp,
                                         scale=SM_SCALE, bias=ebias[:, :])
                return et

            def pv_block(po, prb, jp, et):
                for ct in range(CT):
                    nc.tensor.matmul(
                        po[:, ct, :],
                        vS[:, 2 * jp:2 * jp + 2, ct * 128:(ct + 1) * 128],
                        et[:, :, :], start=(jp == 0), stop=(jp == JP - 1),
                        perf_mode=DR)
                nc.tensor.matmul(
                    prb[:, :], ones3[:, :, :], et[:, :, :],
                    start=(jp == 0), stop=(jp == JP - 1), perf_mode=DR)

            def prefetch_residual(ib):
                bpxs = []
                for ip in range(4):
                    xr = pcr.tile([128, C], F32, tag=f"xr{ip}")
                    nc.sync.dma_start(out=xr, in_=xqt[ib * 4 + ip, :, :])
                    bpx = pcr.tile([128, C], F32, tag=f"bpx{ip}")
                    nc.vector.tensor_tensor(bpx[:, :], xr[:, :], bp_bc[:, :], OP.add)
                    bpxs.append(bpx)
                return bpxs

            def make_proj(ot, bpxs, ib):
                def emit():
                    for ip in range(4):
                        py = ps_y.tile([128, 512], F32, tag="proj")
                        if FP8_P:
                            for k in range(2):
                                nc.tensor.matmul(
                                    py[:, :],
                                    ot[:, 2 * k:2 * k + 2, ip * 128:(ip + 1) * 128],
                                    wp_sb[:, 2 * k:2 * k + 2, :],
                                    start=(k == 0), stop=(k == 1), perf_mode=DR)
                        else:
                            for k in range(CT):
                                nc.tensor.matmul(
                                    py[:, :],
                                    ot[:, k, ip * 128:(ip + 1) * 128],
                                    wp_sb[:, k, :],
                                    start=(k == 0), stop=(k == CT - 1))
                        y2 = pcs.tile([128, C], F32, tag="y2")
                        nc.vector.tensor_tensor(y2[:, :], py[:, :],
                                                bpxs[ip][:, :], OP.add)
                        nc.sync.dma_start(out=yt[ib * 4 + ip, :, :], in_=y2[:, :])
                return emit

            prev_proj = None
            bpxs = prefetch_residual(0)
            for ib in range(IB):
                po = ps_o.tile([128, CT, 512], F32)
                # prb[m, i] = softmax denominator of query i, on every
                # partition m (broadcast rowsum via all-ones lhsT)
                prb = ps_r.tile([128, 512], F32)
                et = scores_block(ib, 0)
                for jp in range(1, JP):
                    et_next = scores_block(ib, jp)
                    pv_block(po, prb, jp - 1, et)
                    et = et_next
                    if jp == 2 and prev_proj is not None:
                        prev_proj()
                    if jp == 3 and ib + 1 < IB:
                        next_bpxs = prefetch_residual(ib + 1)
                pv_block(po, prb, JP - 1, et)
                # normalize during eviction: ot = po * (1/den), both with the
                # query index on the free dim so the broadcast layout matches
                rden = pcr.tile([128, 512], F32, tag="rden")
                nc.vector.reciprocal(rden[:, :], prb[:, :])
                ot = pco.tile([128, CT, 512], F8 if FP8_P else BF16, tag="outT")
                for ct in range(CT):
                    nc.vector.tensor_tensor(ot[:, ct, :], po[:, ct, :],
                                            rden[:, :], OP.mult)
                prev_proj = make_proj(ot, bpxs, ib)
                if ib + 1 < IB:
                    bpxs = next_bpxs
            prev_proj()

        cst.release()
        store.release()

    _split_multi_waits(nc)
    return nc


_PROGRAM = None


def _get_program():
    global _PROGRAM
    if _PROGRAM is None:
        _PROGRAM = build_program()
    return _PROGRAM


def make_in_maps(x, gamma, beta, wq, bq, wk, bk, wv, bv, wp, bp):
    import ml_dtypes
    f32 = lambda a: np.ascontiguousarray(a, dtype=np.float32)
    bf = ml_dtypes.bfloat16
    xr = f32(x).reshape(B, HW, C)
    xr_bf = xr.astype(bf)
    gsel = np.zeros((GROUPS, C), dtype=np.float32)
    for g in range(GROUPS):
        gsel[g, g * GSIZE:(g + 1) * GSIZE] = 1.0
    # gsel2 value 1/(GSIZE*HW) = 2^-16 is exact in bf16; it folds the full
    # group-mean normalization into the PE group-sum matmuls
    gsel2 = np.zeros((128, CT, GROUPS), dtype=ml_dtypes.bfloat16)
    for p in range(128):
        for ct in range(CT):
            gsel2[p, ct, (ct * 128 + p) // GSIZE] = 1.0 / (GSIZE * HW)
    wdt = ml_dtypes.float8_e4m3 if FP8_B else bf
    wl = lambda w, dt=None: np.ascontiguousarray(
        f32(w).astype(dt or wdt).reshape(CT, 128, C).transpose(1, 0, 2))
    # bv folded into bp: attn rows sum to 1, so (o + bv) @ wp + bp
    # == o @ wp + (bv @ wp + bp)
    bp2 = (np.asarray(bv, np.float64) @ np.asarray(wp, np.float64)
           + np.asarray(bp, np.float64))
    common = {
        "wq": wl(wq), "wk": wl(wk), "wv": wl(wv),
        "wp": wl(wp, ml_dtypes.float8_e4m3 if FP8_P else bf),
        "bq": f32(bq).reshape(C, 1), "bk": f32(bk).reshape(C, 1),
        "bp": f32(bp2),
        "gamma": f32(gamma).reshape(C, 1), "beta": f32(beta).reshape(C, 1),
        "gsel": gsel, "gsel2": gsel2,
    }
    in_maps = []
    for c in range(N_CORES):
        b, h = c // 2, c % 2
        m = dict(common)
        if h == 0:
            xp = xr_bf[b]
        else:
            xp = np.concatenate([xr_bf[b, HALF:], xr_bf[b, :HALF]], axis=0)
        # [HW, C] -> [CT, 128, HW] channel-major (full transpose on host)
        m["xt"] = np.ascontiguousarray(
            xp.reshape(HW, CT, 128).transpose(1, 2, 0))
        m["xq"] = np.ascontiguousarray(xr[b, h * HALF:(h + 1) * HALF])
        in_maps.append(m)
    return in_maps


def kernel(x, gamma, beta, wq, bq, wk, bk, wv, bv, wp, bp, _trace=False):
    nc = _get_program()
    in_maps = make_in_maps(x, gamma, beta, wq, bq, wk, bk, wv, bv, wp, bp)
    res = run_bass_kernel_spmd(nc, in_maps, list(range(N_CORES)), trace=_trace)
    out = np.empty((B, HW, C), dtype=np.float32)
    for c in range(N_CORES):
        b, h = c // 2, c % 2
        out[b, h * HALF:(h + 1) * HALF] = res.results[c]["y"]
    if _trace:
        kernel._last_result = res
    return out.reshape(B, H, W, C)


# revision 42
# speedup vs baseline: 1.1997x; 1.1997x over previous
"""AttentionBlock (GroupNorm + single-head 4096x4096 attention + proj + residual)
on 8 Trainium2 NeuronCores.

Sharding: core c = 2*b + h handles image b (of 4), query-half h (of 2).
Each core:
  - receives x pre-transposed to channel-major bf16 (host-side, free),
  - GroupNorm statistics on the otherwise-idle PE: group-select matmuls
    accumulate sum(x)/sum(x^2) into PSUM while x streams in (DVE squares
    each tile); the PSUM eviction's accum_out finishes the reduction,
  - computes kT/v for the full image and qT for its 2048 query rows,
  - all GEMMs (qkv / scores / PV / rowsum / proj) run as fp8e4m3
    DoubleRow matmuls (2x PE throughput).  Softmax without max
    subtraction; exp is computed as exp(s*scale - 3) so the fp8 et
    values stay within e4m3 range (scores are within +-7, e4m3 max
    finite is 240); the -3 offset cancels in the num/den division.
    The softmax denominator comes from a broadcast all-ones DoubleRow
    matmul ([128,2,128] lhsT), landing it on every partition so the
    normalization fuses into the po->ot eviction (one reciprocal + mult),
  - phase C is software-pipelined: scores/exp run one key-pair ahead of
    PV/rowsum, and the projection of i-block ib-1 is deferred into ib's
    jp=2 slot so the PE never waits on the scalar engine,
  - bias + residual for its rows (bv is folded into bp on the host since
    softmax rows sum to 1).
No collectives; k/v compute is duplicated across the half-pair (~10% FLOPs).

Precision: fp8e4m3 GEMM operands everywhere with fp32 PSUM accumulation,
fp32 GroupNorm statistics, fp32 softmax normalization, fp32 residual.
Measured ~5e-3 max rel err vs fp32 (gate is 2e-2).
"""

import sys

sys.path.insert(0, "/opt/trn_rl_repo")

import numpy as np  # noqa: E402

import bass_rust  # noqa: E402
import concourse.bass as bass  # noqa: E402
import concourse.mybir as mybir  # noqa: E402
import concourse.tile as tile  # noqa: E402
from concourse.vector_clock import ScopedClock  # noqa: E402
from concourse.bass_utils import run_bass_kernel_spmd  # noqa: E402

F32 = mybir.dt.float32
BF16 = mybir.dt.bfloat16
F8 = mybir.dt.float8e4
AF = mybir.ActivationFunctionType
OP = mybir.AluOpType
DR = mybir.MatmulPerfMode.DoubleRow

B, H, W, C = 4, 64, 64, 512
HW = H * W            # 4096 positions per image
HALF = HW // 2        # 2048 query rows per core
GROUPS = 32
GSIZE = C // GROUPS   # 16 channels per group
EPS = 1e-5
SM_SCALE = 1.0 / float(np.sqrt(C))
EB = 3.0              # exp bias: et = exp(s*SM - EB), cancels in num/den
N_CORES = 8
FP8_B = True          # fp8 DoubleRow GEMMs in phase B (else bf16)
FP8_P = True          # fp8 DoubleRow projection (else bf16)
CT = C // 128         # 4 channel partition-tiles
JT = HW // 128        # 32 position partition-tiles
JC = HW // 512        # 8 position chunks (kT/v build)
QC = HALF // 512      # 4 query chunks (qT build)
IB = HALF // 512      # 4 query i-blocks (attention)
JP = JT // 2          # 16 key-tile pairs (fp8 DoubleRow)


# --- workaround: walrus in this container rejects instructions carrying more
# than one sync-wait command.  Move extra waits onto same-engine NOPs placed
# immediately before the instruction (engine program order makes this exact).
def _split_multi_waits(nc, max_waits=1):
    n = 0
    for f in nc.m.functions:
        for bb in f.blocks:
            newlist = []
            for inst in bb.instructions:
                si = inst.sync_info
                waits = list(si.on_wait) if si is not None else []
                if len(waits) > max_waits:
                    n += 1
                    for k, wt in enumerate(waits[:-max_waits]):
                        nop = bass_rust.InstNoOp(
                            name=f"{inst.name}-sw{k}", engine=inst.engine)
                        nop.sync_info = mybir.SyncInfo(on_wait=[wt], on_update=[])
                        newlist.append(nop)
                    inst.sync_info = mybir.SyncInfo(
                        on_wait=waits[-max_waits:], on_update=list(si.on_update))
                newlist.append(inst)
            bb.instructions[:] = newlist
    return n


def _split_drain_and_barrier(self, tick_clock, wait_clock):
    # same as TileContext._drain_and_barrier but with the tail drain's waits
    # split onto single-wait NOPs (same walrus limitation as above).
    drain_inst = self.nc.sync.drain()
    wait_clock.add_sem_waits(
        drain_inst.ins, ScopedClock({None: tick_clock.global_clock}))
    mi = drain_inst.ins
    waits = list(mi.sync_info.on_wait) if mi.sync_info is not None else []
    if len(waits) > 1:
        mi.sync_info.on_wait = []
        for wt in waits:
            wi = self.nc.sync.nop(nofuse=True, hint="tail_drain_wait")
            wi.ins.sync_info = mybir.SyncInfo(on_wait=[wt], on_update=[])
    self.nc.all_engine_barrier()
    assert self.sems is not None
    popped = self.nc._tile_sem_poison_stack.pop()
    assert popped is self._sem_poison
    self.nc.clear_and_free_semaphores(list(self.sems.allocated().values()))
    self.nc.all_engine_barrier()


tile.TileContext._drain_and_barrier = _split_drain_and_barrier


def build_program():
    nc = bass.Bass()

    # xt is channel-major bf16, pre-transposed on the host: xt[ct, p, j] =
    # x[j, ct*128+p] with rows permuted per-core so the query half is always
    # positions [0, HALF).  Attention is position-order invariant over keys.
    xtd = nc.dram_tensor("xt", [CT, 128, HW], BF16, kind="ExternalInput")
    xq = nc.dram_tensor("xq", [HALF, C], F32, kind="ExternalInput")
    # weights host-cast in [cin%128, cin//128, cout] layout
    WB = F8 if FP8_B else BF16
    wqd = nc.dram_tensor("wq", [128, CT, C], WB, kind="ExternalInput")
    wkd = nc.dram_tensor("wk", [128, CT, C], WB, kind="ExternalInput")
    wvd = nc.dram_tensor("wv", [128, CT, C], WB, kind="ExternalInput")
    wpd = nc.dram_tensor("wp", [128, CT, C], F8 if FP8_P else BF16, kind="ExternalInput")
    bqd = nc.dram_tensor("bq", [C, 1], F32, kind="ExternalInput")
    bkd = nc.dram_tensor("bk", [C, 1], F32, kind="ExternalInput")
    # bv is folded into bp on the host: softmax rows sum to 1, so
    # attn @ (v + bv) = attn @ v + bv, and (o + bv) @ wp + bp =
    # o @ wp + (bv @ wp + bp).
    bpd = nc.dram_tensor("bp", [C], F32, kind="ExternalInput")
    gamd = nc.dram_tensor("gamma", [C, 1], F32, kind="ExternalInput")
    betd = nc.dram_tensor("beta", [C, 1], F32, kind="ExternalInput")
    gseld = nc.dram_tensor("gsel", [GROUPS, C], F32, kind="ExternalInput")
    # gsel2[p, ct, g] = 1/(GSIZE*HW) where channel ct*128+p belongs to group g
    gsel2d = nc.dram_tensor("gsel2", [128, CT, GROUPS], BF16, kind="ExternalInput")
    yd = nc.dram_tensor("y", [HALF, C], F32, kind="ExternalOutput")

    xqt = xq[:, :].rearrange("(t p) c -> t p c", p=128)   # [16,128,512]
    yt = yd[:, :].rearrange("(t p) c -> t p c", p=128)    # [16,128,512]

    with tile.TileContext(nc) as tc:
        # ---------------- persistent storage + constants ----------------
        store = tc.alloc_tile_pool(name="store", bufs=1)
        kT = store.tile([128, CT, HW], F8)        # kT[c%128, c//128, j]
        vS = store.tile([128, JT, C], F8)         # v[j%128, j//128, c]
        qT = store.tile([128, CT, HALF], F8)      # qT[c%128, c//128, i]
        # x^T in bf16, two tiles per channel-tile (query half first); tile
        # granularity matches the DMA so consumers start per-chunk
        xTs = [[store.tile([128, HW // 2], BF16, tag=f"xT{ct}_{t}",
                           name=f"xT{ct}_{t}") for t in range(2)]
               for ct in range(CT)]
        WB_SB = F8 if FP8_B else BF16
        wq_sb = store.tile([128, CT, C], WB_SB)
        wk_sb = store.tile([128, CT, C], WB_SB)
        wv_sb = store.tile([128, CT, C], WB_SB)
        wp_sb = store.tile([128, CT, C], F8 if FP8_P else BF16)
        cst = tc.alloc_tile_pool(name="cst", bufs=1)
        gsel = cst.tile([GROUPS, C], F32)
        # gsel2 in bf16: its value 1/(GSIZE*HW) = 2^-16 is exact in bf16
        gsel2 = cst.tile([128, CT, GROUPS], BF16)
        # gsel2 is needed by the first phase-A matmuls; everything else
        # streams after x so the x DMAs own the head of both queues
        nc.gpsimd.dma_start(out=gsel2, in_=gsel2d[:, :, :])
        ones3 = cst.tile([128, 2, 128], F8)
        nc.vector.memset(ones3, 1.0)
        ebias = cst.tile([128, 1], F32)
        nc.vector.memset(ebias, -EB)
        bq_sb = cst.tile([128, CT], F32)
        bk_sb = cst.tile([128, CT], F32)
        gam_sb = cst.tile([128, CT], F32)
        bet_sb = cst.tile([128, CT], F32)
        bp_bc = cst.tile([128, C], F32)
        s_sb = cst.tile([128, CT], F32)   # GN scale per channel
        t_sb = cst.tile([128, CT], F32)   # GN shift per channel

        def late_const_dmas():
            for ct in range(CT):
                nc.sync.dma_start(out=bq_sb[:, ct:ct + 1], in_=bqd[ct * 128:(ct + 1) * 128, :])
                nc.sync.dma_start(out=bk_sb[:, ct:ct + 1], in_=bkd[ct * 128:(ct + 1) * 128, :])
                nc.gpsimd.dma_start(out=gam_sb[:, ct:ct + 1], in_=gamd[ct * 128:(ct + 1) * 128, :])
                nc.gpsimd.dma_start(out=bet_sb[:, ct:ct + 1], in_=betd[ct * 128:(ct + 1) * 128, :])
            nc.gpsimd.dma_start(out=gsel, in_=gseld[:, :])
            nc.sync.dma_start(out=bp_bc, in_=bpd[:].partition_broadcast(128))

        # -------- phase A: DMA x^T (host-transposed) + GroupNorm stats ------
        # Group sums of x and x^2 are computed on the (otherwise idle) PE:
        # gsel2^T @ xT accumulates [group, position-chunk] partial sums in
        # PSUM (position chunks fold into the same 512 columns), and the
        # accum_out of the PSUM eviction does the final column reduction.
        # The DVE squares each x tile; gsel2 carries the 1/(GSIZE*HW).
        with tc.tile_pool(name="pa_ps", bufs=1, space="PSUM") as pa_ps, \
             tc.tile_pool(name="pa_bc", bufs=2, space="PSUM") as pa_bc, \
             tc.tile_pool(name="pa_scr", bufs=2) as pa_scr, \
             tc.tile_pool(name="pa_small", bufs=1) as pas:
            gx = pa_ps.tile([GROUPS, 512], F32, tag="gx")
            gx2 = pa_ps.tile([GROUPS, 512], F32, tag="gx2")
            mv2 = pas.tile([GROUPS, 2], F32)
            # arrival order: sync streams ct0,ct1; gpsimd streams ct2,ct3
            order = [(0, 0), (2, 0), (0, 1), (2, 1), (1, 0), (3, 0), (1, 1), (3, 1)]
            for ct, t in order:
                eng = nc.sync if ct < 2 else nc.gpsimd
                eng.dma_start(out=xTs[ct][t],
                              in_=xtd[ct, :, t * 2048:(t + 1) * 2048])
            # weights + small consts queue behind x on both engines
            for i, (wd, wsb) in enumerate(
                    ((wqd, wq_sb), (wkd, wk_sb), (wvd, wv_sb), (wpd, wp_sb))):
                (nc.sync, nc.gpsimd)[i % 2].dma_start(out=wsb, in_=wd[:, :, :])
            late_const_dmas()
            prev = None
            for idx, (ct, t) in enumerate(order):
                for c4 in range(4):
                    nc.tensor.matmul(
                        gx[:, :], gsel2[:, ct, :],
                        xTs[ct][t][:, c4 * 512:(c4 + 1) * 512],
                        start=(idx == 0 and c4 == 0),
                        stop=(idx == 7 and c4 == 3))
                x2 = pa_scr.tile([128, 2048], BF16, tag="x2")
                nc.vector.tensor_tensor(x2[:, :], xTs[ct][t], xTs[ct][t], OP.mult)
                if prev is not None:
                    pct, px2, pidx = prev
                    for c4 in range(4):
                        nc.tensor.matmul(
                            gx2[:, :], gsel2[:, pct, :],
                            px2[:, c4 * 512:(c4 + 1) * 512],
                            start=(pidx == 0 and c4 == 0), stop=False)
                prev = (ct, x2, idx)
            pct, px2, pidx = prev
            for c4 in range(4):
                nc.tensor.matmul(gx2[:, :], gsel2[:, pct, :],
                                 px2[:, c4 * 512:(c4 + 1) * 512],
                                 start=False, stop=(c4 == 3))
            # evict with accum_out: mv2 = (E[x], E[x^2]) per group
            scr32 = pas.tile([GROUPS, 2, 512], F32)
            nc.scalar.activation(scr32[:, 0, :], gx[:, :], AF.Copy,
                                 accum_out=mv2[:, 0:1])
            nc.scalar.activation(scr32[:, 1, :], gx2[:, :], AF.Copy,
                                 accum_out=mv2[:, 1:2])
            var = pas.tile([GROUPS, 1], F32)
            nc.vector.tensor_mul(var[:, :], mv2[:, 0:1], mv2[:, 0:1])
            nc.vector.tensor_sub(var[:, :], mv2[:, 1:2], var[:, :])
            epst = pas.tile([GROUPS, 1], F32)
            nc.vector.memset(epst, EPS)
            sd = pas.tile([GROUPS, 1], F32)
            nc.scalar.activation(sd[:, :], var[:, :], AF.Sqrt, bias=epst[:, :])
            nc.vector.reciprocal(mv2[:, 1:2], sd[:, :])
            # broadcast group (mean, rstd) to channels, then s/t (batched)
            bc_all = pas.tile([128, CT, 2], F32)
            for ct in range(CT):
                pbc = pa_bc.tile([128, 2], F32, tag="bcast")
                nc.tensor.matmul(pbc[:, :], gsel[:, ct * 128:(ct + 1) * 128],
                                 mv2[:, :], start=True, stop=True)
                nc.scalar.activation(bc_all[:, ct, :], pbc[:, :], AF.Copy)
            nc.vector.tensor_mul(s_sb[:, :], gam_sb[:, :], bc_all[:, :, 1])
            tmp = pas.tile([128, CT], F32)
            nc.vector.tensor_mul(tmp[:, :], bc_all[:, :, 0], s_sb[:, :])
            nc.vector.tensor_sub(t_sb[:, :], bet_sb[:, :], tmp[:, :])

        # ---------------- phase B: normalize + K,V (and Q) GEMMs ------------
        # FP8_B: fp8 DoubleRow GEMMs; else bf16 with 4-step contraction.
        def bmm(out, lhs_tile, lhs_cols, rhs_tile, rhs_cols):
            if FP8_B:
                for k in range(2):
                    nc.tensor.matmul(
                        out, lhs_tile[:, 2 * k:2 * k + 2, lhs_cols],
                        rhs_tile[:, 2 * k:2 * k + 2, rhs_cols],
                        start=(k == 0), stop=(k == 1), perf_mode=DR)
            else:
                for k in range(CT):
                    nc.tensor.matmul(
                        out, lhs_tile[:, k, lhs_cols], rhs_tile[:, k, rhs_cols],
                        start=(k == 0), stop=(k == CT - 1))

        def qkv_chunk(pb, pb_ps, jc):
            hnT = pb.tile([128, CT, 512], F8 if FP8_B else BF16, tag="hnT")
            for ct in range(CT):
                # hnT = s * xT + t  (per-channel; channels on partitions)
                nc.vector.tensor_scalar(
                    hnT[:, ct, :],
                    xTs[ct][jc // 4][:, (jc % 4) * 512:(jc % 4 + 1) * 512],
                    s_sb[:, ct:ct + 1], t_sb[:, ct:ct + 1], OP.mult, OP.add)
            for ct in range(CT):
                pk = pb_ps.tile([128, 512], F32, tag="qkv")
                bmm(pk[:, :], wk_sb, slice(ct * 128, (ct + 1) * 128),
                    hnT, slice(None))
                nc.scalar.activation(
                    kT[:, ct, jc * 512:(jc + 1) * 512], pk[:, :],
                    AF.Identity, bias=bk_sb[:, ct:ct + 1])
            if jc < QC:   # rows [0, HALF) are the query rows
                for ct in range(CT):
                    pq = pb_ps.tile([128, 512], F32, tag="qkv")
                    bmm(pq[:, :], wq_sb, slice(ct * 128, (ct + 1) * 128),
                        hnT, slice(None))
                    nc.scalar.activation(
                        qT[:, ct, jc * 512:(jc + 1) * 512], pq[:, :],
                        AF.Identity, bias=bq_sb[:, ct:ct + 1])
            for jp in range(4):
                pv = pb_ps.tile([128, 512], F32, tag="qkv")
                bmm(pv[:, :], hnT, slice(jp * 128, (jp + 1) * 128),
                    wv_sb, slice(None))
                # bv is folded into bp on the host (softmax rows sum to 1)
                nc.vector.tensor_copy(vS[:, jc * 4 + jp, :], pv[:, :])

        with tc.tile_pool(name="pb_sb", bufs=3) as pb, \
             tc.tile_pool(name="pb_ps", bufs=6, space="PSUM") as pb_ps:
            for jc in range(JC):
                qkv_chunk(pb, pb_ps, jc)

        # ---------------- phase C: attention + projection + residual --------
        # scores/PV/rowsum in fp8 DoubleRow (2 k-tiles per matmul, 2x rate)
        with tc.tile_pool(name="pc_sb", bufs=4) as pcs, \
             tc.tile_pool(name="pc_et", bufs=2) as pce, \
             tc.tile_pool(name="pc_res", bufs=2) as pcr, \
             tc.tile_pool(name="pc_o", bufs=2) as pco, \
             tc.tile_pool(name="ps_s", bufs=2, space="PSUM") as ps_s, \
             tc.tile_pool(name="ps_o", bufs=1, space="PSUM") as ps_o, \
             tc.tile_pool(name="ps_r", bufs=1, space="PSUM") as ps_r, \
             tc.tile_pool(name="ps_y", bufs=1, space="PSUM") as ps_y:
            # Software-pipelined: scores/exp for key-pair jp are emitted one
            # iteration ahead of the PV/rowsum that consume them, so the PE
            # never waits on the scalar engine's exp.  The projection of
            # i-block ib-1 is deferred into ib's jp=2 slot so the normalize
            # (vector) runs while the PE streams the next block's scores.
            def scores_block(ib, jp):
                et = pce.tile([128, 2, 512], F8, tag="exp")
                for jj in range(2):
                    j = 2 * jp + jj
                    pss = ps_s.tile([128, 512], F32, tag="scores")
                    nc.tensor.matmul(
                        pss[:, :], kT[:, 0:2, j * 128:(j + 1) * 128],
                        qT[:, 0:2, ib * 512:(ib + 1) * 512],
                        start=True, stop=False, perf_mode=DR)
                    nc.tensor.matmul(
                        pss[:, :], kT[:, 2:4, j * 128:(j + 1) * 128],
                        qT[:, 2:4, ib * 512:(ib + 1) * 512],
                        start=False, stop=True, perf_mode=DR)
                    nc.scalar.activation(et[:, jj, :], pss[:, :], AF.Exp,
                                         scale=SM_SCALE, bias=ebias[:, :])
                return et

            def pv_block(po, prb, jp, et):
                for ct in range(CT):
                    nc.tensor.matmul(
                        po[:, ct, :],
                        vS[:, 2 * jp:2 * jp + 2, ct * 128:(ct + 1) * 128],
                        et[:, :, :], start=(jp == 0), stop=(jp == JP - 1),
                        perf_mode=DR)
                nc.tensor.matmul(
                    prb[:, :], ones3[:, :, :], et[:, :, :],
                    start=(jp == 0), stop=(jp == JP - 1), perf_mode=DR)

            def prefetch_residual(ib):
                bpxs = []
                for ip in range(4):
                    xr = pcr.tile([128, C], F32, tag=f"xr{ip}")
                    nc.sync.dma_start(out=xr, in_=xqt[ib * 4 + ip, :, :])
                    bpx = pcr.tile([128, C], F32, tag=f"bpx{ip}")
                    nc.vector.tensor_tensor(bpx[:, :], xr[:, :], bp_bc[:, :], OP.add)
                    bpxs.append(bpx)
                return bpxs

            def make_proj(ot, bpxs, ib):
                def emit():
                    for ip in range(4):
                        py = ps_y.tile([128, 512], F32, tag="proj")
                        if FP8_P:
                            for k in range(2):
                                nc.tensor.matmul(
                                    py[:, :],
                                    ot[:, 2 * k:2 * k + 2, ip * 128:(ip + 1) * 128],
                                    wp_sb[:, 2 * k:2 * k + 2, :],
                                    start=(k == 0), stop=(k == 1), perf_mode=DR)
                        else:
                            for k in range(CT):
                                nc.tensor.matmul(
                                    py[:, :],
                                    ot[:, k, ip * 128:(ip + 1) * 128],
                                    wp_sb[:, k, :],
                                    start=(k == 0), stop=(k == CT - 1))
                        y2 = pcs.tile([128, C], F32, tag="y2")
                        nc.vector.tensor_tensor(y2[:, :], py[:, :],
                                                bpxs[ip][:, :], OP.add)
                        nc.sync.dma_start(out=yt[ib * 4 + ip, :, :], in_=y2[:, :])
                return emit

            prev_proj = None
            bpxs = prefetch_residual(0)
            for ib in range(IB):
                po = ps_o.tile([128, CT, 512], F32)
                # prb[m, i] = softmax denominator of query i, on every
                # partition m (broadcast rowsum via all-ones lhsT)
                prb = ps_r.tile([128, 512], F32)
                et = scores_block(ib, 0)
                for jp in range(1, JP):
                    et_next = scores_block(ib, jp)
                    pv_block(po, prb, jp - 1, et)
                    et = et_next
                    if jp == 2 and prev_proj is not None:
                        prev_proj()
                    if jp == 3 and ib + 1 < IB:
                        next_bpxs = prefetch_residual(ib + 1)
                pv_block(po, prb, JP - 1, et)
                # normalize during eviction: ot = po * (1/den), both with the
                # query index on the free dim so the broadcast layout matches
                rden = pcr.tile([128, 512], F32, tag="rden")
                nc.vector.reciprocal(rden[:, :], prb[:, :])
                ot = pco.tile([128, CT, 512], F8 if FP8_P else BF16, tag="outT")
                for ct in range(CT):
                    nc.vector.tensor_tensor(ot[:, ct, :], po[:, ct, :],
                                            rden[:, :], OP.mult)
                prev_proj = make_proj(ot, bpxs, ib)
                if ib + 1 < IB:
                    bpxs = next_bpxs
            prev_proj()

        cst.release()
        store.release()

    _split_multi_waits(nc)
    return nc


_PROGRAM = None


def _get_program():
    global _PROGRAM
    if _PROGRAM is None:
        _PROGRAM = build_program()
    return _PROGRAM


def make_in_maps(x, gamma, beta, wq, bq, wk, bk, wv, bv, wp, bp):
    import ml_dtypes
    f32 = lambda a: np.ascontiguousarray(a, dtype=np.float32)
    bf = ml_dtypes.bfloat16
    xr = f32(x).reshape(B, HW, C)
    xr_bf = xr.astype(bf)
    gsel = np.zeros((GROUPS, C), dtype=np.float32)
    for g in range(GROUPS):
        gsel[g, g * GSIZE:(g + 1) * GSIZE] = 1.0
    # gsel2 value 1/(GSIZE*HW) = 2^-16 is exact in bf16; it folds the full
    # group-mean normalization into the PE group-sum matmuls
    gsel2 = np.zeros((128, CT, GROUPS), dtype=ml_dtypes.bfloat16)
    for p in range(128):
        for ct in range(CT):
            gsel2[p, ct, (ct * 128 + p) // GSIZE] = 1.0 / (GSIZE * HW)
    wdt = ml_dtypes.float8_e4m3 if FP8_B else bf
    wl = lambda w, dt=None: np.ascontiguousarray(
        f32(w).astype(dt or wdt).reshape(CT, 128, C).transpose(1, 0, 2))
    # bv folded into bp: attn rows sum to 1, so (o + bv) @ wp + bp
    # == o @ wp + (bv @ wp + bp)
    bp2 = (np.asarray(bv, np.float64) @ np.asarray(wp, np.float64)
           + np.asarray(bp, np.float64))
    common = {
        "wq": wl(wq), "wk": wl(wk), "wv": wl(wv),
        "wp": wl(wp, ml_dtypes.float8_e4m3 if FP8_P else bf),
        "bq": f32(bq).reshape(C, 1), "bk": f32(bk).reshape(C, 1),
        "bp": f32(bp2),
        "gamma": f32(gamma).reshape(C, 1), "beta": f32(beta).reshape(C, 1),
        "gsel": gsel, "gsel2": gsel2,
    }
    in_maps = []
    for c in range(N_CORES):
        b, h = c // 2, c % 2
        m = dict(common)
        if h == 0:
            xp = xr_bf[b]
        else:
            xp = np.concatenate([xr_bf[b, HALF:], xr_bf[b, :HALF]], axis=0)
        # [HW, C] -> [CT, 128, HW] channel-major (full transpose on host)
        m["xt"] = np.ascontiguousarray(
            xp.reshape(HW, CT, 128).transpose(1, 2, 0))
        m["xq"] = np.ascontiguousarray(xr[b, h * HALF:(h + 1) * HALF])
        in_maps.append(m)
    return in_maps


def kernel(x, gamma, beta, wq, bq, wk, bk, wv, bv, wp, bp, _trace=False):
    nc = _get_program()
    in_maps = make_in_maps(x, gamma, beta, wq, bq, wk, bk, wv, bv, wp, bp)
    res = run_bass_kernel_spmd(nc, in_maps, list(range(N_CORES)), trace=_trace)
    out = np.empty((B, HW, C), dtype=np.float32)
    for c in range(N_CORES):
        b, h = c // 2, c % 2
        out[b, h * HALF:(h + 1) * HALF] = res.results[c]["y"]
    if _trace:
        kernel._last_result = res
    return out.reshape(B, H, W, C)
